# revision 1
# baseline (speedup 1.0000x reference)
"""Trainium2 Bass kernel for nn_AutoregressiveDecoder (gnn_message_passing).

Math restructuring (derived from the reference):
  - The per-row conv acts as identity on rows >= i, so with Ahat = max(adj, I),
    CS[i,u] = sum_{v<i} Ahat[v,u] (cumulative column sums), deg_i = rsqrt(CS[i]),
    row i of the output reduces to:
      scores_i[u<i] = tanh(deg_i(u) * (Ahat @ (deg_i^2 * relu(Yt_i)^T r_i))[u]) / 2
      scores_i[i]   = tanh(q_i . q_i) / 2
    where Yt_i = Z1^T (D_i Ahat[:i,:i]),  Z1 = z @ W1[:128],
      q_i = relu(Z1[i] + W1[128]) @ W2,  r_i = W2 @ q_i.
  - Row-parallel across 8 cores, core c handles rows i = c, c+8, ..., c+248
    (interleaved for load balance). Per-core selection flows through one-hot
    input matrices so the compiled program is identical on all cores (SPMD).
  - Per-row heavy matmuls run in bf16 (validated: rel err ~3e-5); the x=0.5zz^T
    term and all degree/tanh math stay fp32.
  - Triangular masks and the transpose identity are built on device via
    affine_select (no DMA).
Host glue: shard/replicate inputs, gather 8x[256,32] score columns + x,
return x + supp + supp.T.
"""

import numpy as np

N = 256
DIN = 128
H1 = 256
H2 = 128
NCORES = 8
NPC = N // NCORES  # 32 rows per core

_PROGRAM = None
LAST_RESULTS = None
TRACE = False
TRACE_KW = {}


def _build_program():
    import concourse.bacc as bacc
    import concourse.mybir as mybir
    from concourse import tile

    F32 = mybir.dt.float32
    BF16 = mybir.dt.bfloat16
    AF = mybir.ActivationFunctionType
    ALU = mybir.AluOpType

    nc = bacc.Bacc()

    blob_d = nc.dram_tensor("blob", [128, 512], F32, kind="ExternalInput")
    blob2_d = nc.dram_tensor("blob2", [128, 2304], BF16, kind="ExternalInput")
    outc_d = nc.dram_tensor("outc", [N, NPC], F32, kind="ExternalOutput")
    x_d = nc.dram_tensor("xout", [N, N], F32, kind="ExternalOutput")

    with tile.TileContext(nc) as tc, tc.tile_pool(name="persist", bufs=1) as P:
        # ---------------- input load: small f32 chunk + bf16 chunk ----------------
        blob = P.tile([128, 512], F32, tag="blob", name="blob")
        blob2 = P.tile([128, 2304], BF16, tag="blob2", name="blob2")
        nc.sync.dma_start(blob[:], blob_d[:])     # z, eye, oc, mc
        nc.sync.dma_start(blob2[:], blob2_d[:])   # ut, adj, lt, w1, w2 (bf16)
        zr = [blob[:, 0:128], blob[:, 128:256]]
        ident = blob[:, 256:384]
        ut = [blob2[:, 0:256], blob2[:, 256:512]]
        adjb = [blob2[:, 512:768], blob2[:, 768:1024]]
        lt = [blob2[:, 1024:1280], blob2[:, 1280:1536]]
        w1bb = blob2[0:1, 1792:2048]
        w1ab = P.tile([128, H1], BF16, tag="w1ab", name="w1ab")
        nc.gpsimd.tensor_copy(w1ab[:], blob2[:, 1536:1792])
        w2hb = [P.tile([128, H2], BF16, tag=f"w2hb{b}", name=f"w2hb{b}") for b in range(2)]
        oc = [blob[:, 384 + b * 32:384 + (b + 1) * 32] for b in range(2)]
        mc = [blob[:, 448 + b * 32:448 + (b + 1) * 32] for b in range(2)]
        ocb = [P.tile([128, NPC], BF16, tag=f"ocb{b}", name=f"ocb{b}") for b in range(2)]
        for b in range(2):
            nc.gpsimd.tensor_copy(w2hb[b][:], blob2[:, 2048 + b * 128:2048 + (b + 1) * 128])
            nc.gpsimd.tensor_copy(ocb[b][:], oc[b][:])

        # ---------------- on-device constants ----------------
        ones_col = P.tile([128, 1], F32, tag="ones_col", name="ones_col")
        nc.vector.memset(ones_col[:], 1.0)
        onesb = P.tile([1, N], BF16, tag="onesb", name="onesb")
        nc.vector.memset(onesb[:], 1.0)
        identb = P.tile([128, 128], BF16, tag="identb", name="identb")
        nc.gpsimd.tensor_copy(identb[:], ident[:])

        # persistent intermediates
        zt = P.tile([128, N], F32, tag="zt", name="zt")              # z^T [k, node]
        ztb = P.tile([128, N], BF16, tag="ztb", name="ztb")
        ahb = [P.tile([128, N], BF16, tag=f"ahb{b}", name=f"ahb{b}") for b in range(2)]
        z1b = [P.tile([128, H1], BF16, tag=f"z1b{b}", name=f"z1b{b}") for b in range(2)]
        rbt = [P.tile([128, N], BF16, tag=f"rbt{b}", name=f"rbt{b}") for b in range(2)]
        qt = P.tile([128, N], F32, tag="qt", name="qt")              # Q^T [d, node]
        qtb = P.tile([128, N], BF16, tag="qtb", name="qtb")
        w2tb = P.tile([128, H1], BF16, tag="w2tb", name="w2tb")      # W2^T [d, h]
        r_t = [P.tile([128, H1], BF16, tag=f"r{b}", name=f"r{b}") for b in range(2)]
        rcb = [P.tile([128, NPC], BF16, tag=f"rcb{b}", name=f"rcb{b}") for b in range(2)]
        ddct = [P.tile([128, NPC], F32, tag=f"ddct{b}", name=f"ddct{b}") for b in range(2)]
        degc = [P.tile([128, NPC], F32, tag=f"degc{b}", name=f"degc{b}") for b in range(2)]
        degcb = [P.tile([128, NPC], BF16, tag=f"degcb{b}", name=f"degcb{b}") for b in range(2)]
        tqh = [P.tile([128, 1], F32, tag=f"tqh{b}", name=f"tqh{b}") for b in range(2)]
        sprime = [P.tile([128, NPC], BF16, tag=f"sprime{b}", name=f"sprime{b}") for b in range(2)]
        fin_th = [P.tile([128, NPC], F32, tag=f"finth{b}", name=f"finth{b}") for b in range(2)]
        for b in range(2):
            nc.vector.memset(sprime[b][:], 0.0)

        with tc.tile_pool(name="pre_ps", bufs=4, space="PSUM") as PS:
            # Ahat = max(adj, I) in bf16
            for b in range(2):
                d0, d1 = b * 128, (1 - b) * 128
                nc.vector.tensor_max(ahb[b][:, d0:d0 + 128], adjb[b][:, d0:d0 + 128], identb[:])
                nc.vector.tensor_copy(ahb[b][:, d1:d1 + 128], adjb[b][:, d1:d1 + 128])
            # CS[i,u]; rec = 1/max(CS,1); dd = rec*LT; deg = sqrt(rec)
            rec = [P.tile([128, N], F32, tag=f"rec{b}", name=f"rec{b}") for b in range(2)]
            dd = [P.tile([128, N], F32, tag=f"dd{b}", name=f"dd{b}") for b in range(2)]
            ltf = [P.tile([128, N], F32, tag=f"ltf{b}", name=f"ltf{b}") for b in range(2)]
            for b in range(2):
                nc.vector.tensor_copy(ltf[b][:], lt[b][:])
            for ib in range(2):
                ps = PS.tile([128, N], F32, tag="ps", name="ps")
                for vb in range(2):
                    nc.tensor.matmul(ps[:], ut[vb][:, ib * 128:(ib + 1) * 128], ahb[vb][:],
                                     start=(vb == 0), stop=(vb == 1))
                nc.vector.reciprocal(rec[ib][:], ps[:])
                nc.gpsimd.tensor_scalar_min(rec[ib][:], rec[ib][:], 1.0)
                nc.vector.tensor_mul(dd[ib][:], rec[ib][:], ltf[ib][:])

            # DEGC[u, j] = sqrt(rec[i_j, u]) * (u < i_j); sqrt via quake-rsqrt on DVE
            I32 = mybir.dt.int32
            for ub in range(2):
                ps = PS.tile([128, NPC], F32, tag="ps", name="ps")
                for ib in range(2):
                    nc.tensor.matmul(ps[:], rec[ib][:, ub * 128:(ub + 1) * 128], oc[ib][:],
                                     start=(ib == 0), stop=(ib == 1))
                x = P.tile([128, NPC], F32, tag=f"recc{ub}", name=f"recc{ub}")
                nc.vector.tensor_copy(x[:], ps[:])
                t1 = P.tile([128, NPC], F32, tag=f"qt1_{ub}", name=f"qt1_{ub}")
                nc.scalar.activation(t1[:], x[:], AF.Sqrt)
                nc.vector.tensor_mul(degc[ub][:], t1[:], mc[ub][:])

            # DDCT[v, j] = DD[i_j, v]
            for vb in range(2):
                dps = PS.tile([128, NPC], F32, tag="ps", name="ps")
                for ib in range(2):
                    nc.tensor.matmul(dps[:], dd[ib][:, vb * 128:(vb + 1) * 128], oc[ib][:],
                                     start=(ib == 0), stop=(ib == 1))
                nc.vector.tensor_copy(ddct[vb][:], dps[:])

            # z^T via PE transpose
            for b in range(2):
                ps = PS.tile([128, 128], F32, tag="ps", name="ps")
                nc.tensor.transpose(ps[:], zr[b][:], ident[:])
                nc.scalar.activation(zt[:, b * 128:(b + 1) * 128], ps[:], AF.Copy)
                nc.vector.tensor_copy(ztb[:, b * 128:(b + 1) * 128], ps[:])

            # Z1 = z @ W1a -> bf16
            for b in range(2):
                ps = PS.tile([128, H1], F32, tag="ps", name="ps")
                nc.tensor.matmul(ps[:], ztb[:, b * 128:(b + 1) * 128], w1ab[:], start=True, stop=True)
                nc.vector.tensor_copy(z1b[b][:], ps[:])

            # ZB^T = W1^T [z|1]^T ; RBT = relu(ZB^T)
            for hb in range(2):
                ps = PS.tile([128, N], F32, tag="ps", name="ps")
                nc.tensor.matmul(ps[:], w1ab[:, hb * 128:(hb + 1) * 128], ztb[:], start=True, stop=False)
                nc.tensor.matmul(ps[:], w1bb[0:1, hb * 128:(hb + 1) * 128], onesb[0:1, :], start=False, stop=True)
                nc.scalar.activation(rbt[hb][:], ps[:], AF.Relu)

            # Q^T = W2^T relu(ZB)^T
            qps = PS.tile([128, N], F32, tag="ps", name="ps")
            for hb in range(2):
                nc.tensor.matmul(qps[:], w2hb[hb][:], rbt[hb][:], start=(hb == 0), stop=(hb == 1))
            nc.scalar.activation(qt[:], qps[:], AF.Copy)
            nc.vector.tensor_copy(qtb[:], qps[:])

            # W2^T via PE transpose
            for b in range(2):
                ps = PS.tile([128, 128], BF16, tag="psb", name="psb")
                nc.tensor.transpose(ps[:], w2hb[b][:], identb[:])
                nc.scalar.activation(w2tb[:, b * 128:(b + 1) * 128], ps[:], AF.Copy)

            # R = Q @ W2^T
            for ib in range(2):
                ps = PS.tile([128, H1], F32, tag="ps", name="ps")
                nc.tensor.matmul(ps[:], qtb[:, ib * 128:(ib + 1) * 128], w2tb[:], start=True, stop=True)
                nc.scalar.activation(r_t[ib][:], ps[:], AF.Copy)

            # RC[h, j] = sum_i R[i,h] OC[i,j] -> bf16
            for hb in range(2):
                ps = PS.tile([128, NPC], F32, tag="ps", name="ps")
                for ib in range(2):
                    nc.tensor.matmul(ps[:], r_t[ib][:, hb * 128:(hb + 1) * 128], ocb[ib][:],
                                     start=(ib == 0), stop=(ib == 1))
                nc.vector.tensor_copy(rcb[hb][:], ps[:])


        # ---------------- per-row loop ----------------
        # Two-stage software pipeline over row-pairs: stage A emits the scaled
        # rhs + Yt matmuls + fused relu for pair k; stage B (emitted after pair
        # k+1's matmuls so PE's in-order queue never stalls on ACT) emits the
        # t-matvecs + sprime writes for pair k, and the incremental U fold.
        with tc.tile_pool(name="loop_ps", bufs=2, space="PSUM") as LPS, \
             tc.tile_pool(name="loop_sb", bufs=3) as LSB:
            def stage_a(jp):
                js = (2 * jp, 2 * jp + 1)
                mjs = [min(8 * (j + 1), N) for j in js]
                ps = LPS.tile([128, 4, N], F32, tag="yt", name="yt")
                ftt = LSB.tile([128, 4, N], BF16, tag="ft", name="ft")
                for q, j in enumerate(js):
                    mj = mjs[q]
                    nvb = 1 if mj <= 128 else 2
                    sah = []
                    for vb in range(nvb):
                        sz = min(mj, 128) if vb == 0 else mj - 128
                        s = LSB.tile([128, N], BF16, tag=f"sah{q}{vb}", name=f"sah{q}{vb}")
                        nc.vector.tensor_scalar_mul(s[0:sz, 0:mj], ahb[vb][0:sz, 0:mj],
                                                    degc[vb][0:sz, j:j + 1])
                        sah.append((s, sz))
                    # keep each PSUM accumulation group contiguous (vb inner):
                    # interleaved open groups on one tile miscompute on HW
                    for hb in range(2):
                        for vb in range(nvb):
                            s, sz = sah[vb]
                            nc.tensor.matmul(ps[:, 2 * q + hb, 0:mj],
                                             z1b[vb][0:sz, hb * 128:(hb + 1) * 128],
                                             s[0:sz, 0:mj], start=(vb == 0), stop=(vb == nvb - 1))
                nc.scalar.activation(ftt[:, 0:4, 0:mjs[1]], ps[:, 0:4, 0:mjs[1]], AF.Relu)
                return (jp, js, mjs, ftt)

            def stage_b(state):
                jp, js, mjs, ftt = state
                for q, j in enumerate(js):
                    mj = mjs[q]
                    nvb = 1 if mj <= 128 else 2
                    for vb in range(nvb):
                        sz = min(mj, 128) if vb == 0 else mj - 128
                        tcol = LPS.tile([128, 1], F32, tag="tc", name="tc")
                        for hb in range(2):
                            nc.tensor.matmul(tcol[0:sz, :], ftt[:, 2 * q + hb, vb * 128:vb * 128 + sz],
                                             rcb[hb][:, j:j + 1], start=(hb == 0), stop=(hb == 1))
                        nc.vector.tensor_mul(sprime[vb][0:sz, j:j + 1], tcol[0:sz, :],
                                             ddct[vb][0:sz, j:j + 1])
                if jp % 4 == 3:
                    jc = jp // 4
                    c0 = jc * 8
                    for ub in range(2):
                        ps_u = LPS.tile([128, 8], F32, tag="u", name="u")
                        for vb in range(2):
                            nc.tensor.matmul(ps_u[:], ahb[vb][:, ub * 128:(ub + 1) * 128],
                                             sprime[vb][:, c0:c0 + 8],
                                             start=(vb == 0), stop=(vb == 1))
                        w = P.tile([128, 8], F32, tag=f"w{ub}_{jc}", name=f"w{ub}_{jc}")
                        nc.vector.tensor_mul(w[:], ps_u[:], degc[ub][:, c0:c0 + 8])
                        nc.scalar.activation(fin_th[ub][:, c0:c0 + 8], w[:], AF.Tanh)

            pending = None
            for jp in range(NPC // 2):
                state = stage_a(jp)
                if pending is not None:
                    stage_b(pending)
                pending = state
            stage_b(pending)

        with tc.tile_pool(name="tail_ps", bufs=2, space="PSUM") as TPS:
            # X = 0.5 z z^T -> dram
            xsb = P.tile([128, 2, N], F32, tag="xsb", name="xsb")
            for b in range(2):
                ps = TPS.tile([128, N], F32, tag="xps", name="xps")
                nc.tensor.matmul(ps[:], zt[:, b * 128:(b + 1) * 128], zt[:], start=True, stop=True)
                nc.vector.tensor_scalar_mul(xsb[:, b, :], ps[:], 0.5)
            for b in range(2):
                nc.sync.dma_start(x_d[b * 128:(b + 1) * 128, :], xsb[:, b, :])
            # qq[i] = sum_d Q^T[d,i]^2 ; tqh = 0.5*tanh(qq)
            sq = P.tile([128, N], F32, tag="sq", name="sq")
            nc.vector.tensor_mul(sq[:], qt[:], qt[:])
            for ib in range(2):
                ps = TPS.tile([128, 1], F32, tag="qqps", name="qqps")
                nc.tensor.matmul(ps[:], sq[:, ib * 128:(ib + 1) * 128], ones_col[:], start=True, stop=True)
                th = P.tile([128, 1], F32, tag=f"qth{ib}", name=f"qth{ib}")
                nc.scalar.activation(th[:], ps[:], AF.Tanh)
                nc.vector.tensor_scalar_mul(tqh[ib][:], th[:], 0.5)

        # ---------------- final combine (one DMA) ----------------
        fin = P.tile([128, 2, NPC], F32, tag="fin", name="fin")
        for ub in range(2):
            th = fin_th[ub]
            dg = P.tile([128, NPC], F32, tag=f"dg{ub}", name=f"dg{ub}")
            nc.vector.tensor_scalar_mul(dg[:], oc[ub][:], tqh[ub][:])
            nc.vector.tensor_scalar_mul(th[:], th[:], 0.5)
            nc.vector.tensor_add(fin[:, ub, :], th[:], dg[:])
        for b in range(2):
            nc.sync.dma_start(outc_d[b * 128:(b + 1) * 128, :], fin[:, b, :])

    nc.finalize()
    return nc


def _get_program():
    global _PROGRAM
    if _PROGRAM is None:
        _PROGRAM = _build_program()
    return _PROGRAM


def kernel(z, adj, W1, W2):
    global LAST_RESULTS
    from concourse.bass_utils import run_bass_kernel_spmd

    z = np.ascontiguousarray(np.asarray(z, np.float32))
    adj = np.ascontiguousarray(np.asarray(adj, np.float32))
    W1 = np.ascontiguousarray(np.asarray(W1, np.float32))
    W2 = np.ascontiguousarray(np.asarray(W2, np.float32))

    idx = np.arange(N)
    nc = _get_program()
    in_maps = []
    for c in range(NCORES):
        ii = np.arange(c, N, NCORES)
        OC = np.zeros((N, NPC), np.float32)
        OC[ii, np.arange(NPC)] = 1.0
        MC = (idx[:, None] < ii[None, :]).astype(np.float32)
        import ml_dtypes
        bf = ml_dtypes.bfloat16
        blob = np.zeros((128, 512), np.float32)
        blob[:, 0:128] = z[0:128]
        blob[:, 128:256] = z[128:256]
        blob[:, 256:384] = np.eye(128, dtype=np.float32)
        blob[:, 384:416] = OC[0:128]
        blob[:, 416:448] = OC[128:256]
        blob[:, 448:480] = MC[0:128]
        blob[:, 480:512] = MC[128:256]
        UTf = (idx[:, None] < idx[None, :])
        LTf = UTf.T
        blob2 = np.zeros((128, 2304), bf)
        blob2[:, 0:256] = UTf[0:128].astype(bf)
        blob2[:, 256:512] = UTf[128:256].astype(bf)
        blob2[:, 512:768] = adj[0:128].astype(bf)
        blob2[:, 768:1024] = adj[128:256].astype(bf)
        blob2[:, 1024:1280] = LTf[0:128].astype(bf)
        blob2[:, 1280:1536] = LTf[128:256].astype(bf)
        blob2[:, 1536:1792] = W1[0:128].astype(bf)
        blob2[0, 1792:2048] = W1[128].astype(bf)
        blob2[:, 2048:2176] = W2[0:128].astype(bf)
        blob2[:, 2176:2304] = W2[128:256].astype(bf)
        in_maps.append({"blob": blob, "blob2": blob2})
    res = run_bass_kernel_spmd(nc, in_maps, list(range(NCORES)),
                               trace=TRACE, **TRACE_KW)
    LAST_RESULTS = res
    supp = np.zeros((N, N), np.float32)
    for c in range(NCORES):
        supp[np.arange(c, N, NCORES), :] = res.results[c]["outc"].T
    x = res.results[0]["xout"]
    return (x + supp + supp.T).astype(np.float32)



# revision 14
# speedup vs baseline: 1.3884x; 1.3884x over previous
"""Trainium2 Bass kernel for nn_AutoregressiveDecoder (gnn_message_passing).

Math (derived from the reference):
  With Ahat = max(adj, I), CS[i,u] = sum_{v<i} Ahat[v,u], deg_i = CS[i]^-1/2,
  row i of supp reduces to
    supp[i,u<i] = 0.5*tanh(deg_i(u) * (Ahat @ (deg_i^2 * relu(Yt_i)^T r_i))[u])
    supp[i,i]   = 0.5*tanh(q_i . q_i)
  where Yt_i = Z1^T (D_i Ahat), Z1 = z @ W1[:128],
    q_i = relu(Z1[i] + W1[128]) @ W2,  r_i = W2 @ q_i.
  Output = 0.5 z z^T + supp + supp^T.

Mapping (per core, rows i = c, c+8, ..., c+248; SPMD via one-hot inputs):
  - Rows processed in 11 groups of g rows (g in {8,4,2}) sharing one PSUM
    tile: stage_a emits 2*nvb matmuls of free-size g*mjg (amortizes the
    ~180ns isolated-matmul cost), relu'd to bf16 (split ACT/DVE).
  - t_j = relu(Yt_j)^T r_j uses a diagonal-expanded rc (lhsT = 32-col
    window with only col j nonzero -> LDWEIGHTS is 32 cols); out row j of a
    persistent [32,256] PSUM tile accumulates across all rows.
  - deg via quake-rsqrt on DVE (int shift/sub + 1 Newton step): the kernel
    then needs only one ACT table set (relu/tanh/copy) -> 1 table load.
  - U-fold (Ahat @ sprime), deg scale, tanh, diag term and the 0.5 z z^T
    column strip are computed once at the end; one [128,128] f32 DMA out.
Host glue: builds per-core one-hots/masks and pre-transposed bf16 layouts,
gathers 8x[256,64] strips, returns x + supp + supp.T.
"""

import numpy as np

N = 256
NCORES = 8
NPC = N // NCORES  # 32 rows per core

# (start_row, g) per group; mjg = 8*(start+g)
GROUPS = [(0, 8), (8, 4), (12, 4)] + [(j, 2) for j in range(16, 32, 2)]
DVE_RELU = {0, 1}  # groups whose relu runs on Vector instead of Scalar

_PROGRAM = None
LAST_RESULTS = None
TRACE = False
TRACE_KW = {}

QUAKE_MAGIC = 0x5F3759DF


def _build_program():
    import concourse.bacc as bacc
    import concourse.mybir as mybir
    from concourse import tile

    F32 = mybir.dt.float32
    BF16 = mybir.dt.bfloat16
    I32 = mybir.dt.int32
    AF = mybir.ActivationFunctionType
    ALU = mybir.AluOpType

    nc = bacc.Bacc()

    b1_d = nc.dram_tensor("b1", [128, 1024], BF16, kind="ExternalInput")
    b2_d = nc.dram_tensor("b2", [128, 1600], BF16, kind="ExternalInput")
    out_d = nc.dram_tensor("outp", [128, 128], F32, kind="ExternalOutput")

    with tile.TileContext(nc) as tc, tc.tile_pool(name="persist", bufs=1) as P:
        b1 = P.tile([128, 1024], BF16, tag="b1", name="b1")
        b2 = P.tile([128, 1600], BF16, tag="b2", name="b2")
        nc.sync.dma_start(b1[:], b1_d[:])
        nc.sync.dma_start(b2[:], b2_d[:])
        ahb = [b1[:, 0:256], b1[:, 256:512]]
        ut = [b1[:, 512:768], b1[:, 768:1024]]
        ztb = b2[:, 0:256]
        w1ab = b2[:, 256:512]
        w1bb = b2[0:1, 512:768]
        w2h = [b2[:, 768:896], b2[:, 896:1024]]
        w2tb = b2[:, 1024:1280]
        ocb = [b2[:, 1280:1312], b2[:, 1312:1344]]
        mcb = b2[:, 1344:1408]                      # [128, 2*32]
        identb = b2[:, 1408:1536]
        zoc = b2[:, 1536:1568]                      # z rows for this core, [128d, 32]

        onesb = P.tile([1, 256], BF16, tag="onesb", name="onesb")
        nc.vector.memset(onesb[:], 1.0)
        onescol = P.tile([128, 1], BF16, tag="onescol", name="onescol")
        nc.vector.memset(onescol[:], 1.0)
        zero32 = P.tile([128, 32], BF16, tag="zero32", name="zero32")
        nc.vector.memset(zero32[:], 0.0)
        magic = P.tile([128, 64], I32, tag="magic", name="magic")
        nc.vector.memset(magic[:], QUAKE_MAGIC)
        rczf = [P.tile([128, 1056], BF16, tag=f"rczf{hb}", name=f"rczf{hb}")
                for hb in range(2)]
        for hb in range(2):
            nc.vector.memset(rczf[hb][:], 0.0)

        # persistent SBUF intermediates
        cs_sb = P.tile([128, 2, 256], BF16, tag="cs_sb", name="cs_sb")
        z1b = [P.tile([128, 256], BF16, tag=f"z1b{nb}", name=f"z1b{nb}") for nb in range(2)]
        rbt = [P.tile([128, 256], BF16, tag=f"rbt{hb}", name=f"rbt{hb}") for hb in range(2)]
        qtb = P.tile([128, 256], BF16, tag="qtb", name="qtb")
        sqb = P.tile([128, 256], BF16, tag="sqb", name="sqb")
        rsb = [P.tile([128, 256], BF16, tag=f"rsb{nb}", name=f"rsb{nb}") for nb in range(2)]
        rc_sb = P.tile([128, 2, 32], BF16, tag="rc_sb", name="rc_sb")
        csc_sb = P.tile([128, 64], F32, tag="csc_sb", name="csc_sb")
        qi32 = P.tile([128, 64], I32, tag="qi32", name="qi32")
        y0 = P.tile([128, 64], F32, tag="y0", name="y0")
        yt2 = P.tile([128, 64], F32, tag="yt2", name="yt2")
        mcf = P.tile([128, 64], F32, tag="mcf", name="mcf")
        degf = P.tile([128, 64], F32, tag="degf", name="degf")
        degcb = P.tile([128, 2, 32], BF16, tag="degcb", name="degcb")
        degcT = P.tile([32, 256], BF16, tag="degcT", name="degcT")
        ddctT = P.tile([32, 256], F32, tag="ddctT", name="ddctT")
        tqh = P.tile([128, 2], F32, tag="tqh", name="tqh")
        dg = P.tile([128, 2, 32], F32, tag="dg", name="dg")
        fin = P.tile([128, 2, 64], F32, tag="fin", name="fin")
        spT = P.tile([32, 256], BF16, tag="spT", name="spT")
        spc = P.tile([128, 2, 32], BF16, tag="spc", name="spc")
        wt = P.tile([128, 64], F32, tag="wt", name="wt")
        tht = P.tile([128, 64], F32, tag="tht", name="tht")

        with tc.tile_pool(name="pre_ps", bufs=2, space="PSUM") as PS:
            # CS[i,u] = sum_w UT[w,i] Ahat[w,u]
            for ib in range(2):
                ps = PS.tile([128, 256], F32, tag="ps", name="ps")
                for wb in range(2):
                    nc.tensor.matmul(ps[:], ut[wb][:, ib * 128:(ib + 1) * 128], ahb[wb][:],
                                     start=(wb == 0), stop=(wb == 1))
                nc.vector.tensor_copy(cs_sb[:, ib, :], ps[:])

            # Z1 = z @ W1a   (lhsT = z^T block, rhs = W1a)
            for nb in range(2):
                ps = PS.tile([128, 256], F32, tag="ps", name="ps")
                nc.tensor.matmul(ps[:], ztb[:, nb * 128:(nb + 1) * 128], w1ab[:],
                                 start=True, stop=True)
                nc.vector.tensor_copy(z1b[nb][:], ps[:])

            # rbt = relu(W1^T [z|1]^T)
            for hb in range(2):
                ps = PS.tile([128, 256], F32, tag="ps", name="ps")
                nc.tensor.matmul(ps[:], w1ab[:, hb * 128:(hb + 1) * 128], ztb[:],
                                 start=True, stop=False)
                nc.tensor.matmul(ps[:], w1bb[:, hb * 128:(hb + 1) * 128], onesb[:],
                                 start=False, stop=True)
                nc.scalar.activation(rbt[hb][:], ps[:], AF.Relu)

            # CSC[u, (ub,j)] = CS[i_j, u]  (select rows of CS via one-hots)
            csc = PS.tile([128, 2, 32], F32, tag="csc", name="csc")
            for ub in range(2):
                for ib in range(2):
                    nc.tensor.matmul(csc[:, ub, :],
                                     cs_sb[:, ib, ub * 128:(ub + 1) * 128], ocb[ib][:],
                                     start=(ib == 0), stop=(ib == 1))
            nc.vector.tensor_copy(csc_sb[:], csc[:, :, :])

            # quake rsqrt: deg = CS^-1/2 (exact-int CS; 1 Newton step)
            nc.vector.tensor_single_scalar(qi32[:], csc_sb[:].bitcast(I32), 1,
                                           ALU.arith_shift_right)
            nc.vector.tensor_sub(y0[:].bitcast(I32), magic[:], qi32[:])
            nc.vector.tensor_mul(yt2[:], y0[:], y0[:])
            nc.vector.tensor_mul(yt2[:], yt2[:], csc_sb[:])
            nc.vector.tensor_scalar(yt2[:], yt2[:], -0.5, 1.5, ALU.mult, ALU.add)
            nc.vector.tensor_mul(degf[:], y0[:], yt2[:])
            nc.vector.tensor_copy(mcf[:], mcb[:])
            nc.vector.tensor_mul(degf[:], degf[:], mcf[:])
            nc.vector.tensor_copy(degcb[:, :, :],
                                  degf[:].rearrange("p (u j) -> p u j", u=2))

            # Q^T = W2^T relu(ZB)^T  -> qtb [d, n]
            ps = PS.tile([128, 256], F32, tag="ps", name="ps")
            for hb in range(2):
                nc.tensor.matmul(ps[:], w2h[hb][:], rbt[hb][:],
                                 start=(hb == 0), stop=(hb == 1))
            nc.vector.tensor_copy(qtb[:], ps[:])
            nc.vector.tensor_mul(sqb[:], qtb[:], qtb[:])

            # R = Q @ W2^T  -> rsb [n-block, h]
            for nb in range(2):
                ps = PS.tile([128, 256], F32, tag="ps", name="ps")
                nc.tensor.matmul(ps[:], qtb[:, nb * 128:(nb + 1) * 128], w2tb[:],
                                 start=True, stop=True)
                nc.vector.tensor_copy(rsb[nb][:], ps[:])

            # rc[h, j] = R[i_j, h]
            rcps = PS.tile([128, 2, 32], F32, tag="csc", name="csc")
            for hb in range(2):
                for nb in range(2):
                    nc.tensor.matmul(rcps[:, hb, :],
                                     rsb[nb][:, hb * 128:(hb + 1) * 128], ocb[nb][:],
                                     start=(nb == 0), stop=(nb == 1))
            nc.vector.tensor_copy(rc_sb[:, :, :], rcps[:, :, :])
            # diagonal-expand rc into rczf (col j of window j nonzero)
            for hb in range(2):
                dst = rczf[hb][:, 0:1056].rearrange("p (j k) -> p j k", k=33)[:, :, 0:1]
                nc.vector.tensor_copy(dst, rc_sb[:, hb, :].unsqueeze(2))

            # qq[n] = |q_n|^2 ; tqh = tanh(qq); dg = 0.5 * oc * tqh
            qq = PS.tile([128, 2, 32], F32, tag="csc", name="qq")
            for nb in range(2):
                nc.tensor.matmul(qq[:, nb, 0:1], sqb[:, nb * 128:(nb + 1) * 128],
                                 onescol[:], start=True, stop=True)
            nc.scalar.activation(tqh[:].rearrange("p (u j) -> p u j", u=2),
                                 qq[:, :, 0:1], AF.Tanh)
            for ib in range(2):
                nc.vector.tensor_scalar(dg[:, ib, :], ocb[ib][:], tqh[:, ib:ib + 1],
                                        0.5, ALU.mult, ALU.mult)

            # X strip: 0.5 * z z^T columns for this core
            for ub in range(2):
                ps = PS.tile([128, 2, 32], F32, tag="csc", name="csc")
                nc.tensor.matmul(ps[:, 0, :], ztb[:, ub * 128:(ub + 1) * 128], zoc[:],
                                 start=True, stop=True)
                nc.vector.tensor_scalar_mul(fin[:, ub, 32:64], ps[:, 0, :], 0.5)

            # degcT[j, u] = degc[u, j]; ddctT = degcT^2
            for ub in range(2):
                pst = PS.tile([32, 128], BF16, tag="pst", name="pst")
                nc.tensor.transpose(pst[:], degcb[:, ub, :], identb[:])
                nc.vector.tensor_copy(degcT[:, ub * 128:(ub + 1) * 128], pst[:])
            nc.vector.tensor_mul(ddctT[:], degcT[:], degcT[:])

        # ---------------- grouped row loop ----------------
        with tc.tile_pool(name="tp", bufs=1, space="PSUM") as TP:
            t_rows = TP.tile([128, 256], F32, tag="t_rows", name="t_rows")
            # clear has_written across [0:32, 0:256] (zero weights)
            nc.tensor.matmul(t_rows[0:32, 0:256], zero32[:], ahb[0][:],
                             start=True, stop=False, skip_group_check=True)

            with tc.tile_pool(name="loop_ps", bufs=2, space="PSUM") as LPS, \
                 tc.tile_pool(name="loop_sb", bufs=3) as LSB:

                def stage_a(gi):
                    j0, g = GROUPS[gi]
                    mjg = 8 * (j0 + g)
                    gm = g * mjg
                    nvb = 1 if mjg <= 128 else 2
                    yt = LPS.tile([128, 2, 512], F32, tag="yt", name="yt")
                    s = LSB.tile([128, 2, 512], BF16, tag="s", name="s")
                    for vb in range(nvb):
                        sz = min(mjg, 128) if vb == 0 else mjg - 128
                        for q in range(g):
                            j = j0 + q
                            nc.vector.tensor_scalar_mul(
                                s[0:sz, vb, q * mjg:(q + 1) * mjg],
                                ahb[vb][0:sz, 0:mjg],
                                degf[0:sz, vb * 32 + j:vb * 32 + j + 1])
                    for hb in range(2):
                        for vb in range(nvb):
                            sz = min(mjg, 128) if vb == 0 else mjg - 128
                            nc.tensor.matmul(yt[:, hb, 0:gm],
                                             z1b[vb][0:sz, hb * 128:(hb + 1) * 128],
                                             s[0:sz, vb, 0:gm],
                                             start=(vb == 0), stop=(vb == nvb - 1))
                    return (gi, j0, g, mjg, yt)

                def stage_b(state):
                    gi, j0, g, mjg, yt = state
                    gm = g * mjg
                    last = (gi == len(GROUPS) - 1)
                    ftt = LSB.tile([128, 2, 512], BF16, tag="ftt", name="ftt")
                    if gi in DVE_RELU:
                        nc.vector.tensor_scalar_max(ftt[:, :, 0:gm], yt[:, :, 0:gm], 0.0)
                    else:
                        nc.scalar.activation(ftt[:, :, 0:gm], yt[:, :, 0:gm], AF.Relu)
                    for q in range(g):
                        j = j0 + q
                        for hb in range(2):
                            nc.tensor.matmul(t_rows[0:32, 0:mjg],
                                             rczf[hb][:, j * 32:j * 32 + 32],
                                             ftt[:, hb, q * mjg:(q + 1) * mjg],
                                             start=False,
                                             stop=(last and q == g - 1 and hb == 1),
                                             skip_group_check=True)

                pending = None
                for gi in range(len(GROUPS)):
                    st = stage_a(gi)
                    if pending is not None:
                        stage_b(pending)
                    pending = st
                stage_b(pending)

            # ---------------- tail (t_rows pool still open) ----------------
            with tc.tile_pool(name="tail_ps", bufs=2, space="PSUM") as TPS:
                nc.vector.tensor_mul(spT[:], t_rows[0:32, :], ddctT[:])
                for vb in range(2):
                    pst2 = TPS.tile([128, 32], BF16, tag="pst2", name="pst2")
                    nc.tensor.transpose(pst2[:], spT[:, vb * 128:(vb + 1) * 128],
                                        identb[0:32, 0:32])
                    nc.vector.tensor_copy(spc[:, vb, :], pst2[:])
                up = TPS.tile([128, 2, 32], F32, tag="up", name="up")
                for ub in range(2):
                    for vb in range(2):
                        nc.tensor.matmul(up[:, ub, :],
                                         ahb[vb][:, ub * 128:(ub + 1) * 128],
                                         spc[:, vb, :],
                                         start=(vb == 0), stop=(vb == 1))
                nc.vector.tensor_mul(wt[:], up[:, :, :].rearrange("p u j -> p (u j)"),
                                     degf[:])
                nc.scalar.activation(tht[:], wt[:], AF.Tanh)
                # fin = 0.5 * tanh + dg  (the 0.5 scales the tanh OUTPUT)
                nc.vector.scalar_tensor_tensor(
                    fin[:, :, 0:32],
                    tht[:].rearrange("p (u j) -> p u j", u=2),
                    0.5, dg[:, :, :], ALU.mult, ALU.add)
        nc.sync.dma_start(out_d[:], fin[:, :, :].rearrange("p u j -> p (u j)"))

    nc.finalize()
    return nc


def _get_program():
    global _PROGRAM
    if _PROGRAM is None:
        _PROGRAM = _build_program()
    return _PROGRAM


def kernel(z, adj, W1, W2):
    global LAST_RESULTS
    from concourse.bass_utils import run_bass_kernel_spmd
    import ml_dtypes

    bf = ml_dtypes.bfloat16
    z = np.asarray(z, np.float32)
    adj = np.asarray(adj, np.float32)
    W1 = np.asarray(W1, np.float32)
    W2 = np.asarray(W2, np.float32)

    idx = np.arange(N)
    Ahat = np.maximum(adj, np.eye(N, dtype=np.float32))
    UT = (idx[:, None] < idx[None, :]).astype(np.float32)
    zt = z.T  # [128, 256]

    nc = _get_program()
    in_maps = []
    for c in range(NCORES):
        ii = np.arange(c, N, NCORES)
        OC = np.zeros((N, NPC), np.float32)
        OC[ii, np.arange(NPC)] = 1.0
        MC = (idx[:, None] < ii[None, :]).astype(np.float32)  # [256, 32]

        b1 = np.zeros((128, 1024), bf)
        b1[:, 0:256] = Ahat[0:128].astype(bf)
        b1[:, 256:512] = Ahat[128:256].astype(bf)
        b1[:, 512:768] = UT[0:128].astype(bf)
        b1[:, 768:1024] = UT[128:256].astype(bf)

        b2 = np.zeros((128, 1600), bf)
        b2[:, 0:256] = zt.astype(bf)
        b2[:, 256:512] = W1[0:128].astype(bf)
        b2[0, 512:768] = W1[128].astype(bf)
        b2[:, 768:896] = W2[0:128].astype(bf)
        b2[:, 896:1024] = W2[128:256].astype(bf)
        b2[:, 1024:1280] = W2.T.astype(bf)
        b2[:, 1280:1312] = OC[0:128].astype(bf)
        b2[:, 1312:1344] = OC[128:256].astype(bf)
        b2[:, 1344:1376] = MC[0:128].astype(bf)
        b2[:, 1376:1408] = MC[128:256].astype(bf)
        b2[:, 1408:1536] = np.eye(128, dtype=np.float32).astype(bf)
        b2[:, 1536:1568] = zt[:, ii].astype(bf)
        in_maps.append({"b1": b1, "b2": b2})

    res = run_bass_kernel_spmd(nc, in_maps, list(range(NCORES)),
                               trace=TRACE, **TRACE_KW)
    LAST_RESULTS = res

    supp = np.zeros((N, N), np.float32)
    x = np.zeros((N, N), np.float32)
    for c in range(NCORES):
        ii = np.arange(c, N, NCORES)
        out_r = np.asarray(res.results[c]["outp"], np.float32).reshape(128, 2, 64)
        # supp[i_j, u] where u = ub*128 + p
        supp[ii, :] = out_r[:, :, 0:32].transpose(2, 1, 0).reshape(NPC, N)
        # x[u, i_j]
        x[:, ii] = out_r[:, :, 32:64].transpose(1, 0, 2).reshape(N, NPC)
    return (x + supp + supp.T).astype(np.float32)
